# revision 4
# baseline (speedup 1.0000x reference)
"""AdaptiveSudokuLoss on 8 TRN2 NeuronCores — pure data-parallel.

Full inputs: outputs (65536, 81, 9) f32, targets (65536, 81) int64.
Output: scalar f32 loss.

Math (per cell, 9 logits x_d, no max-subtraction needed for randn inputs):
  e = exp(x); s = sum_d e; logZ = ln s; p = e/s
  CE_sum   = sum_cells (logZ - x_t)
  conf_sum = sum_cells (logZ - sum_d p*x)      [uses sum_d p == 1]
  per constraint type (row/col/box): sum (g-1)^2 = sum g(g-1) - sum_all p + Ngroups
  loss = (1.1*S_logZ - S_xw)/N + 0.5*(S_rgg+S_cgg+S_bgg - 3*S_p + 3*N)/(B*243)
  where S_xw = sum x*(onehot(t) + 0.1*p) folds the CE gather and entropy dot.

Each core processes 8192 samples, emits 6 partial sums as [128, 8] f32;
host combines. No collectives needed.
"""
import numpy as np
import ml_dtypes

import concourse.bass as bass
import concourse.tile as tile_mod
from concourse import mybir
from concourse.bass_utils import run_bass_kernel_spmd
from concourse.vector_clock import ScopedClock

# ---------------------------------------------------------------- tile fix --
# walrus (b16 2026-05-04) rejects >1-2 sem-waits on one CTRL instruction; the
# TileContext tail drain carries one wait per used processor. Redistribute
# them one-per-NOP ahead of the drain.


_nop_counter = [0]


def _split_multi_waits(nc):
    """walrus in this container accepts only one sem-wait per instruction.
    Hoist extra waits onto same-engine NOPs inserted just before."""
    for fn in nc.m.functions:
        for bb in fn.blocks:
            out = []
            changed = False
            for inst in bb.instructions:
                si = inst.sync_info
                if si is not None and len(si.on_wait) > 1:
                    waits = list(si.on_wait)
                    for w in waits[:-1]:
                        _nop_counter[0] += 1
                        n = mybir.InstNoOp(
                            name=f"I-waitsplit-{_nop_counter[0]}", ins=[], outs=[])
                        n.engine = inst.engine
                        n.sync_info = mybir.SyncInfo(on_wait=[w], on_update=[])
                        out.append(n)
                    si.on_wait = waits[-1:]
                    inst.sync_info = si
                    changed = True
                out.append(inst)
            if changed:
                bb.instructions = out


def _patched_drain_and_barrier(self, tick_clock, wait_clock):
    nc = self.nc
    probe = nc.sync.nop()
    wait_clock.add_sem_waits(probe.ins, ScopedClock({None: tick_clock.global_clock}))
    nc.sync.drain()
    nc.all_engine_barrier()
    assert self.sems is not None
    popped = nc._tile_sem_poison_stack.pop()
    assert popped is self._sem_poison
    nc.clear_and_free_semaphores(list(self.sems.allocated().values()))
    nc.all_engine_barrier()
    _split_multi_waits(nc)


tile_mod.TileContext._drain_and_barrier = _patched_drain_and_barrier

# ------------------------------------------------------------------- consts --
B = 65536
NCORES = 8
BS = B // NCORES            # samples per core = 8192
P = 128                     # partitions
SPP = BS // P               # samples per partition = 64
CPP = SPP * 81              # cells per partition = 5184
FPP = CPP * 9               # floats per partition = 46656
NT = 16                     # tiles
TS = SPP // NT              # samples per partition per tile = 4
TC = TS * 81                # cells  = 324
TF = TC * 9                 # floats = 2916

F32 = mybir.dt.float32
BF16 = mybir.dt.bfloat16
ALU = mybir.AluOpType
ACTF = mybir.ActivationFunctionType
AX = mybir.AxisListType

_CACHE = {}


def _build():
    nc = bass.Bass()
    x_ext = nc.declare_dram_parameter("x", [P, FPP], F32, isOutput=False)
    t_ext = nc.declare_dram_parameter("t", [P, CPP], BF16, isOutput=False)
    io_ext = nc.declare_dram_parameter("io9", [P, 9], BF16, isOutput=False)
    out_ext = nc.declare_dram_parameter("out", [P, 8], F32, isOutput=True)

    with tile_mod.TileContext(nc) as tc:
        with (
            tc.tile_pool(name="work", bufs=2) as wp,
            tc.tile_pool(name="pers", bufs=1) as pp,
        ):
            io9 = pp.tile([P, 9], BF16)
            nc.sync.dma_start(io9[:], io_ext[:])
            accL = pp.tile([P, NT], F32)   # sum logZ
            accP = pp.tile([P, NT], F32)   # sum p
            accXW = pp.tile([P, NT], F32)  # sum x*(oh + 0.1 p)
            accR = pp.tile([P, NT], F32)   # sum g(g-1) rows
            accC = pp.tile([P, NT], F32)   # cols
            accB = pp.tile([P, NT], F32)   # boxes

            for k in range(NT):
                xt = wp.tile([P, TF], F32)
                nc.sync.dma_start(xt[:], x_ext[:, k * TF:(k + 1) * TF])
                tt = wp.tile([P, TC], BF16)
                nc.sync.dma_start(tt[:], t_ext[:, k * TC:(k + 1) * TC])

                x3 = xt[:].rearrange("p (c d) -> p c d", d=9)

                et = wp.tile([P, TF], BF16)
                nc.scalar.activation(et[:], xt[:], ACTF.Exp)
                e3 = et[:].rearrange("p (c d) -> p c d", d=9)

                st = wp.tile([P, TC], F32)
                nc.vector.tensor_reduce(st[:], e3, axis=AX.X, op=ALU.add)

                lst = wp.tile([P, TC], F32)
                nc.scalar.activation(lst[:], st[:], ACTF.Ln,
                                     accum_out=accL[:, k:k + 1])
                rt = wp.tile([P, TC], BF16)
                nc.scalar.activation(rt[:], lst[:], ACTF.Exp, scale=-1.0)

                # onehot(t) on gpsimd: oh[c, d] = (io9[d] == t[c])
                oh = wp.tile([P, TF], BF16)
                oh3 = oh[:].rearrange("p (c d) -> p c d", d=9)
                nc.vector.tensor_tensor(
                    oh3,
                    io9[:].unsqueeze(1).broadcast_to([P, TC, 9]),
                    tt[:].unsqueeze(2).broadcast_to([P, TC, 9]),
                    op=ALU.is_equal)

                # p = e * (1/s); accP += sum p
                pt = wp.tile([P, TF], BF16)
                p3 = pt[:].rearrange("p (c d) -> p c d", d=9)
                nc.vector.scalar_tensor_tensor(
                    out=p3, in0=e3, scalar=1.0,
                    in1=rt[:].unsqueeze(2).broadcast_to([P, TC, 9]),
                    op0=ALU.mult, op1=ALU.mult,
                    accum_out=accP[:, k:k + 1])

                # w = oh + 0.1 p   (in place over oh)
                nc.vector.scalar_tensor_tensor(
                    out=oh3, in0=p3, scalar=0.1, in1=oh3,
                    op0=ALU.mult, op1=ALU.add)

                # accXW += sum x*w   (product dumped over xt, dead after)
                nc.vector.scalar_tensor_tensor(
                    out=x3, in0=x3, scalar=1.0, in1=oh3,
                    op0=ALU.mult, op1=ALU.mult,
                    accum_out=accXW[:, k:k + 1])

                # --- constraint group sums from p ---------------------------
                # per sample: flat = s*729 + r*81 + c*9 + d ; c = 3C+j
                p6 = pt[:].rearrange("p (s r C j d) -> p s r C j d",
                                     s=TS, r=9, C=3, j=3, d=9)
                a3 = wp.tile([P, TS * 243], BF16)       # (s, r, C, d)
                a3v = a3[:].rearrange("p (s r C d) -> p s r C d",
                                      s=TS, r=9, C=3, d=9)
                nc.vector.tensor_tensor(a3v, p6[:, :, :, :, 0, :],
                                        p6[:, :, :, :, 1, :], op=ALU.add)
                nc.vector.tensor_tensor(a3v, a3v, p6[:, :, :, :, 2, :],
                                        op=ALU.add)

                # rows: sum over C -> (s, r, d)
                a3c = a3[:].rearrange("p (s r C d) -> p s r C d",
                                      s=TS, r=9, C=3, d=9)
                rg = wp.tile([P, TC], BF16)
                rgv = rg[:].rearrange("p (s r d) -> p s r d", s=TS, r=9, d=9)
                nc.vector.tensor_tensor(rgv, a3c[:, :, :, 0, :],
                                        a3c[:, :, :, 1, :], op=ALU.add)
                nc.vector.tensor_tensor(rgv, rgv, a3c[:, :, :, 2, :],
                                        op=ALU.add)

                # boxes: sum over i (r = 3R+i) -> (s, R, C, d)
                a3r = a3[:].rearrange("p (s R i Cd) -> p s R i Cd",
                                      s=TS, R=3, i=3, Cd=27)
                bg = wp.tile([P, TC], BF16)
                bgv = bg[:].rearrange("p (s R Cd) -> p s R Cd",
                                      s=TS, R=3, Cd=27)
                nc.vector.tensor_tensor(bgv, a3r[:, :, :, 0, :],
                                        a3r[:, :, :, 1, :], op=ALU.add)
                nc.vector.tensor_tensor(bgv, bgv, a3r[:, :, :, 2, :],
                                        op=ALU.add)

                # cols: reduce over r -> (s, c, d)
                pcol = pt[:].rearrange("p (s r c d) -> p s c d r",
                                       s=TS, r=9, c=9, d=9)
                cg = wp.tile([P, TC], F32)
                cgv = cg[:].rearrange("p (s c d) -> p s c d", s=TS, c=9, d=9)
                nc.vector.tensor_reduce(cgv, pcol, axis=AX.X, op=ALU.add)

                # sum g(g-1) per type
                scr = wp.tile([P, TC], BF16)
                nc.vector.scalar_tensor_tensor(
                    out=scr[:], in0=rg[:], scalar=-1.0, in1=rg[:],
                    op0=ALU.add, op1=ALU.mult, accum_out=accR[:, k:k + 1])
                nc.vector.scalar_tensor_tensor(
                    out=scr[:], in0=cg[:], scalar=-1.0, in1=cg[:],
                    op0=ALU.add, op1=ALU.mult, accum_out=accC[:, k:k + 1])
                nc.vector.scalar_tensor_tensor(
                    out=scr[:], in0=bg[:], scalar=-1.0, in1=bg[:],
                    op0=ALU.add, op1=ALU.mult, accum_out=accB[:, k:k + 1])

            ot = pp.tile([P, 8], F32)
            nc.vector.tensor_reduce(ot[:, 0:1], accL[:], axis=AX.X, op=ALU.add)
            nc.vector.tensor_reduce(ot[:, 1:2], accP[:], axis=AX.X, op=ALU.add)
            nc.vector.tensor_reduce(ot[:, 2:3], accXW[:], axis=AX.X, op=ALU.add)
            nc.vector.tensor_reduce(ot[:, 3:4], accR[:], axis=AX.X, op=ALU.add)
            nc.vector.tensor_reduce(ot[:, 4:5], accC[:], axis=AX.X, op=ALU.add)
            nc.vector.tensor_reduce(ot[:, 5:6], accB[:], axis=AX.X, op=ALU.add)
            nc.vector.memset(ot[:, 6:8], 0.0)
            nc.sync.dma_start(out_ext[:], ot[:])
    return nc


def _get_nc():
    if "nc" not in _CACHE:
        _CACHE["nc"] = _build()
    return _CACHE["nc"]


def kernel(outputs: np.ndarray, targets: np.ndarray, _want_results=False,
           **run_kwargs) -> np.ndarray:
    nc = _get_nc()
    io9 = np.broadcast_to(np.arange(9, dtype=ml_dtypes.bfloat16), (P, 9)).copy()
    in_maps = []
    for i in range(NCORES):
        xs = np.ascontiguousarray(
            outputs[i * BS:(i + 1) * BS], dtype=np.float32).reshape(P, FPP)
        ts = np.ascontiguousarray(
            targets[i * BS:(i + 1) * BS]).astype(
                ml_dtypes.bfloat16).reshape(P, CPP)
        in_maps.append({"x": xs, "t": ts, "io9": io9})
    res = run_bass_kernel_spmd(nc, in_maps, core_ids=list(range(NCORES)),
                               **run_kwargs)

    S = np.zeros(8, dtype=np.float64)
    for i in range(NCORES):
        S += res.results[i]["out"].astype(np.float64).sum(axis=0)
    S_logZ, S_p, S_xw, S_r, S_c, S_b = S[0], S[1], S[2], S[3], S[4], S[5]
    N = float(B * 81)
    term1 = (1.1 * S_logZ - S_xw) / N
    csum = (S_r + S_c + S_b) - 3.0 * S_p + 3.0 * N
    loss = term1 + 0.5 * csum / (B * 9.0 * 27.0)
    out = np.float32(loss)
    if _want_results:
        return out, res
    return out


# revision 6
# speedup vs baseline: 1.1419x; 1.1419x over previous
"""AdaptiveSudokuLoss on 8 TRN2 NeuronCores — pure data-parallel.

Full inputs: outputs (65536, 81, 9) f32, targets (65536, 81) int64.
Output: scalar f32 loss.

Host preprocessing: cast x to bf16 and pad the digit axis 9 -> 10 with -100
(exp -> 0, never equals a target, keeps every on-chip run even-length and
4B-aligned so bf16 tensor_tensor ops hit the 2x packed mode).

Math per cell (9 logits x_d; randn inputs need no max-subtraction):
  e = exp(x); s = sum_d e; logZ = ln s; p = e/s
  CE_sum + 0.1*conf_sum = 1.1*sum logZ - sum x*(onehot(t) + 0.1*p)
  constraint: sum over row/col/box groups of (g-1)^2, g = group sum of p
  loss = (1.1*S_logZ - S_xw)/N + 0.5*(S_r+S_c+S_b)/(B*9*27)

Each core processes 8192 samples, emits partial sums as [128, 8] f32;
host combines. No collectives.
"""
import numpy as np
import ml_dtypes

import concourse.bass as bass
import concourse.tile as tile_mod
from concourse import mybir
from concourse.bass_utils import run_bass_kernel_spmd
from concourse.vector_clock import ScopedClock

# ---------------------------------------------------------------- tile fix --
# walrus (b16 2026-05-04) accepts only one sem-wait per instruction; Tile's
# add_semaphores attaches several. Hoist extras onto same-engine NOPs.

_nop_counter = [0]


def _split_multi_waits(nc):
    for fn in nc.m.functions:
        for bb in fn.blocks:
            out = []
            changed = False
            for inst in bb.instructions:
                si = inst.sync_info
                if si is not None and len(si.on_wait) > 1:
                    waits = list(si.on_wait)
                    for w in waits[:-1]:
                        _nop_counter[0] += 1
                        n = mybir.InstNoOp(
                            name=f"I-waitsplit-{_nop_counter[0]}", ins=[], outs=[])
                        n.engine = inst.engine
                        n.sync_info = mybir.SyncInfo(on_wait=[w], on_update=[])
                        out.append(n)
                    si.on_wait = waits[-1:]
                    inst.sync_info = si
                    changed = True
                out.append(inst)
            if changed:
                bb.instructions = out


def _patched_drain_and_barrier(self, tick_clock, wait_clock):
    nc = self.nc
    probe = nc.sync.nop()
    wait_clock.add_sem_waits(probe.ins, ScopedClock({None: tick_clock.global_clock}))
    nc.sync.drain()
    nc.all_engine_barrier()
    assert self.sems is not None
    popped = nc._tile_sem_poison_stack.pop()
    assert popped is self._sem_poison
    nc.clear_and_free_semaphores(list(self.sems.allocated().values()))
    nc.all_engine_barrier()
    _split_multi_waits(nc)


tile_mod.TileContext._drain_and_barrier = _patched_drain_and_barrier

# ------------------------------------------------------------------- consts --
B = 65536
NCORES = 8
BS = B // NCORES            # samples per core = 8192
P = 128                     # partitions
SPP = BS // P               # samples per partition = 64
CPP = SPP * 81              # cells per partition = 5184
D = 10                      # padded digit axis
FPP = CPP * D               # bf16 elems per partition = 51840
NT = 16                     # tiles
TS = SPP // NT              # samples per partition per tile = 4
TC = TS * 81                # cells = 324
TF = TC * D                 # elems = 3240

F32 = mybir.dt.float32
BF16 = mybir.dt.bfloat16
ALU = mybir.AluOpType
ACTF = mybir.ActivationFunctionType
AX = mybir.AxisListType

_CACHE = {}


def _build():
    nc = bass.Bass()
    cm1 = nc.alloc_sbuf_tensor("const-float32-neg1", [128, 1], F32)
    nc.gpsimd.memset(cm1.ap(), -1.0)
    nc.const_aps.aps[(F32, -1.0)] = cm1.ap()
    nc.all_engine_barrier()
    x_ext = nc.declare_dram_parameter("x", [P, FPP], BF16, isOutput=False)
    t_ext = nc.declare_dram_parameter("t", [P, CPP], BF16, isOutput=False)
    io_ext = nc.declare_dram_parameter("io10", [P, D], BF16, isOutput=False)
    out_ext = nc.declare_dram_parameter("out", [P, 8], F32, isOutput=True)

    with tile_mod.TileContext(nc) as tc:
        with (
            tc.tile_pool(name="work", bufs=2) as wp,
            tc.tile_pool(name="pers", bufs=1) as pp,
        ):
            io10 = pp.tile([P, D], BF16)
            nc.sync.dma_start(io10[:], io_ext[:])
            accL = pp.tile([P, NT], F32)   # sum logZ
            accXW = pp.tile([P, NT], F32)  # sum x*(oh + 0.1 p)
            accR = pp.tile([P, NT], F32)   # sum (g-1)^2 rows
            accC = pp.tile([P, NT], F32)   # cols
            accB = pp.tile([P, NT], F32)   # boxes

            for k in range(NT):
                xt = wp.tile([P, TF], BF16)
                nc.sync.dma_start(xt[:], x_ext[:, k * TF:(k + 1) * TF])
                tt = wp.tile([P, TC], BF16)
                nc.sync.dma_start(tt[:], t_ext[:, k * TC:(k + 1) * TC])

                x3 = xt[:].rearrange("p (c d) -> p c d", d=D)

                et = wp.tile([P, TF], BF16)
                nc.scalar.activation(et[:], xt[:], ACTF.Exp)
                e3 = et[:].rearrange("p (c d) -> p c d", d=D)

                st = wp.tile([P, TC], F32)
                nc.vector.tensor_reduce(st[:], e3, axis=AX.X, op=ALU.add)

                lst = wp.tile([P, TC], F32)
                nc.scalar.activation(lst[:], st[:], ACTF.Ln,
                                     accum_out=accL[:, k:k + 1])
                rt = wp.tile([P, TC], BF16)
                nc.scalar.activation(rt[:], lst[:], ACTF.Exp, scale=-1.0)

                # onehot(t): oh[c, d] = (io10[d] == t[c])
                oh = wp.tile([P, TF], BF16)
                oh3 = oh[:].rearrange("p (c d) -> p c d", d=D)
                nc.vector.tensor_tensor(
                    oh3,
                    io10[:].unsqueeze(1).broadcast_to([P, TC, D]),
                    tt[:].unsqueeze(2).broadcast_to([P, TC, D]),
                    op=ALU.is_equal)

                # p = e * (1/s)
                pt = wp.tile([P, TF], BF16)
                p3 = pt[:].rearrange("p (c d) -> p c d", d=D)
                nc.vector.scalar_tensor_tensor(
                    out=p3, in0=e3, scalar=1.0,
                    in1=rt[:].unsqueeze(2).broadcast_to([P, TC, D]),
                    op0=ALU.mult, op1=ALU.mult)

                # w = oh + 0.1 p   (in place over oh)
                nc.vector.scalar_tensor_tensor(
                    out=oh3, in0=p3, scalar=0.1, in1=oh3,
                    op0=ALU.mult, op1=ALU.add)

                # accXW += sum x*w  (product dumped over xt, dead after)
                nc.vector.scalar_tensor_tensor(
                    out=x3, in0=x3, scalar=1.0, in1=oh3,
                    op0=ALU.mult, op1=ALU.mult,
                    accum_out=accXW[:, k:k + 1])

                # --- constraint group sums from p (all runs even, aligned) --
                # per sample: flat = s*810 + r*90 + c*10 + d ; c = 3C+j
                p6 = pt[:].rearrange("p (s r C j d) -> p s r C j d",
                                     s=TS, r=9, C=3, j=3, d=D)
                a3 = wp.tile([P, TS * 270], BF16)       # (s, r, C, d)
                a3v = a3[:].rearrange("p (s r C d) -> p s r C d",
                                      s=TS, r=9, C=3, d=D)
                nc.vector.tensor_tensor(a3v, p6[:, :, :, :, 0, :],
                                        p6[:, :, :, :, 1, :], op=ALU.add)
                nc.vector.tensor_tensor(a3v, a3v, p6[:, :, :, :, 2, :],
                                        op=ALU.add)

                # rows: sum over C -> (s, r, d)   runs of 10
                a3c = a3[:].rearrange("p (s r C d) -> p s r C d",
                                      s=TS, r=9, C=3, d=D)
                rg = wp.tile([P, TS * 90], BF16)
                rgv = rg[:].rearrange("p (s r d) -> p s r d", s=TS, r=9, d=D)
                nc.vector.tensor_tensor(rgv, a3c[:, :, :, 0, :],
                                        a3c[:, :, :, 1, :], op=ALU.add)
                nc.vector.tensor_tensor(rgv, rgv, a3c[:, :, :, 2, :],
                                        op=ALU.add)

                # boxes: sum over i (r = 3R+i) -> (s, R, C, d)  runs of 30
                a3r = a3[:].rearrange("p (s R i Cd) -> p s R i Cd",
                                      s=TS, R=3, i=3, Cd=3 * D)
                bg = wp.tile([P, TS * 90], BF16)
                bgv = bg[:].rearrange("p (s R Cd) -> p s R Cd",
                                      s=TS, R=3, Cd=3 * D)
                nc.vector.tensor_tensor(bgv, a3r[:, :, :, 0, :],
                                        a3r[:, :, :, 1, :], op=ALU.add)
                nc.vector.tensor_tensor(bgv, bgv, a3r[:, :, :, 2, :],
                                        op=ALU.add)

                # cols via r-triples: b3 = sum over i of p rows, runs of 90
                pr = pt[:].rearrange("p (s R i cd) -> p s R i cd",
                                     s=TS, R=3, i=3, cd=9 * D)
                b3 = wp.tile([P, TS * 270], BF16)       # (s, R, c, d)
                b3v = b3[:].rearrange("p (s R cd) -> p s R cd",
                                      s=TS, R=3, cd=9 * D)
                nc.vector.tensor_tensor(b3v, pr[:, :, :, 0, :],
                                        pr[:, :, :, 1, :], op=ALU.add)
                nc.vector.tensor_tensor(b3v, b3v, pr[:, :, :, 2, :],
                                        op=ALU.add)
                # cols: sum over R -> (s, c, d)  runs of 90
                b3r = b3[:].rearrange("p (s R cd) -> p s R cd",
                                      s=TS, R=3, cd=9 * D)
                cg = wp.tile([P, TS * 90], BF16)
                cgv = cg[:].rearrange("p (s cd) -> p s cd", s=TS, cd=9 * D)
                nc.vector.tensor_tensor(cgv, b3r[:, :, 0, :],
                                        b3r[:, :, 1, :], op=ALU.add)
                nc.vector.tensor_tensor(cgv, cgv, b3r[:, :, 2, :],
                                        op=ALU.add)

                # sum (g-1)^2 per type on ScalarE: Square(g*1 + (-1)), accum
                scr = wp.tile([P, TS * 90], BF16)
                nc.scalar.activation(scr[:], rg[:], ACTF.Square, bias=-1.0,
                                     accum_out=accR[:, k:k + 1])
                nc.scalar.activation(scr[:], cg[:], ACTF.Square, bias=-1.0,
                                     accum_out=accC[:, k:k + 1])
                nc.scalar.activation(scr[:], bg[:], ACTF.Square, bias=-1.0,
                                     accum_out=accB[:, k:k + 1])

            ot = pp.tile([P, 8], F32)
            nc.vector.tensor_reduce(ot[:, 0:1], accL[:], axis=AX.X, op=ALU.add)
            nc.vector.tensor_reduce(ot[:, 2:3], accXW[:], axis=AX.X, op=ALU.add)
            nc.vector.tensor_reduce(ot[:, 3:4], accR[:], axis=AX.X, op=ALU.add)
            nc.vector.tensor_reduce(ot[:, 4:5], accC[:], axis=AX.X, op=ALU.add)
            nc.vector.tensor_reduce(ot[:, 5:6], accB[:], axis=AX.X, op=ALU.add)
            nc.vector.memset(ot[:, 1:2], 0.0)
            nc.vector.memset(ot[:, 6:8], 0.0)
            nc.sync.dma_start(out_ext[:], ot[:])
    return nc


def _get_nc():
    if "nc" not in _CACHE:
        _CACHE["nc"] = _build()
    return _CACHE["nc"]


def _prep_x(outputs):
    """(B, 81, 9) f32 -> per-core [128, FPP] bf16 with digit pad -100."""
    xb = np.full((B, 81, D), -100.0, dtype=ml_dtypes.bfloat16)
    xb[:, :, :9] = outputs.astype(ml_dtypes.bfloat16)
    return xb.reshape(NCORES, P, FPP)


def kernel(outputs: np.ndarray, targets: np.ndarray, _want_results=False,
           **run_kwargs) -> np.ndarray:
    nc = _get_nc()
    io10 = np.broadcast_to(np.arange(D, dtype=ml_dtypes.bfloat16), (P, D)).copy()
    xs_all = _prep_x(np.ascontiguousarray(outputs, dtype=np.float32))
    ts_all = np.ascontiguousarray(targets).astype(
        ml_dtypes.bfloat16).reshape(NCORES, P, CPP)
    in_maps = [{"x": xs_all[i], "t": ts_all[i], "io10": io10}
               for i in range(NCORES)]
    res = run_bass_kernel_spmd(nc, in_maps, core_ids=list(range(NCORES)),
                               **run_kwargs)

    S = np.zeros(8, dtype=np.float64)
    for i in range(NCORES):
        S += res.results[i]["out"].astype(np.float64).sum(axis=0)
    S_logZ, S_xw, S_r, S_c, S_b = S[0], S[2], S[3], S[4], S[5]
    N = float(B * 81)
    term1 = (1.1 * S_logZ - S_xw) / N
    loss = term1 + 0.5 * (S_r + S_c + S_b) / (B * 9.0 * 27.0)
    out = np.float32(loss)
    if _want_results:
        return out, res
    return out
